# revision 38
# baseline (speedup 1.0000x reference)
"""Trainium2 Bass kernel for the two-branch KV-cache attention problem.

Math: the reference computes attention over [k_cache_gpu; k_new] (with a causal
mask on the new columns) and separately over k_cache_cpu, then merges the two
partial softmax states in log2-lse space.  That merge is mathematically the
softmax over the union of all kv columns, so we compute ONE softmax over all
4096 + 8192 + 128 = 12416 columns per (batch*head, q) row.  We use the
*unstable* softmax (no row-max subtraction): scores are N(0, sqrt(128)) so
exp stays well inside fp32 range, and masked (-65504) scores underflow to
exactly 0 like the reference.

Sharding: bh (=64) split across 8 cores, 8 bh each; merge is purely local.

Per 512-kv chunk (per bh):
  DMA   : K chunk fp32 -> SBUF [128,(4,128)]    (HWDGE)
          V chunk fp32 -> SBUF bf16 [128,(4,129)] (SWDGE cast) + ones column
  PE    : 4x fp32 transpose K tiles -> PSUM kt [d=128, kv=512]
  DVE   : copy kt PSUM -> SBUF (rounding to float32r when enabled)
  PE    : S = matmul(lhsT=Q^T, rhs=K^T) -> PSUM [q=128, kv=512]
  ACT   : P = exp(S) PSUM -> SBUF bf16
  PE    : 4x bf16 transpose P tiles -> PSUM pt [kv, q]
  ACT/DVE: copy pt PSUM -> SBUF (alternating)
  PE    : 4x matmul(lhsT=P^T tile, rhs=[V|1] bf16) accumulate -> O PSUM [q,129]
Tail per bh: recip(se=O[:,128]) on DVE, scale O[:,0:128], DMA out.

The ones column appended to V makes the PV matmul also produce the softmax
denominator (sum of the exact bf16-rounded P actually used for the numerator).
"""

import numpy as np

BATCH = 2
HEADS = 32
BH = BATCH * HEADS
QS = 128
D = 128
KV_G = 4096
KV_C = 8192
N_CORES = 8
BH_PER_CORE = BH // N_CORES

CHUNK = 512
SUB = CHUNK // 128  # 128-wide subtiles per chunk


def emit_attention(tc, outs, ins, n_bh=BH_PER_CORE, kv_g=KV_G, kv_c=KV_C,
                   s_fp32r=True, bufs_sbuf=3, repeat=1, kt_mode='pe',
                   chunks_g=None, chunks_c=None, no_ktp=False, no_ptp=False,
                   dma_only=False, v_mode='swdge', dma_group=1, k_group=None,
                   k_rings=1):
    """Emit the attention program into TileContext `tc`."""
    from contextlib import ExitStack
    from concourse import masks, mybir

    nc = tc.nc
    f32 = mybir.dt.float32
    f32r = mybir.dt.float32r
    bf16 = mybir.dt.bfloat16
    EXP = mybir.ActivationFunctionType.Exp

    q, k, v = ins['q'], ins['k'], ins['v']
    kg, vg, kc, vc = ins['kg'], ins['vg'], ins['kc'], ins['vc']
    mask = ins['mask']
    y = outs['y']

    assert kv_g % CHUNK == 0 and kv_c % CHUNK == 0

    with ExitStack() as ctx:
        ep = ctx.enter_context

        consts = ep(tc.tile_pool(name="consts", bufs=1))
        kn_pool = ep(tc.tile_pool(name="kn", bufs=bufs_sbuf))
        vt_pool = ep(tc.tile_pool(name="vt", bufs=bufs_sbuf))
        kt_pool = ep(tc.tile_pool(name="kt", bufs=bufs_sbuf))
        p_pool = ep(tc.tile_pool(name="p", bufs=bufs_sbuf))
        pt_pool = ep(tc.tile_pool(name="pt", bufs=bufs_sbuf))
        qt_pool = ep(tc.tile_pool(name="qt", bufs=2))
        o_pool = ep(tc.tile_pool(name="o", bufs=2))
        kt_psum = ep(tc.tile_pool(name="ktp", bufs=2, space="PSUM"))
        s_psum = ep(tc.tile_pool(name="sp", bufs=2, space="PSUM"))
        pt_psum = ep(tc.tile_pool(name="ptp", bufs=2, space="PSUM"))
        o_psum = ep(tc.tile_pool(name="op", bufs=2, space="PSUM"))

        u16 = bf16  # 2-byte carrier for bit-exact half transposes
        ident_f32 = consts.tile([128, 128], f32)
        ident_bf16 = consts.tile([128, 128], bf16)
        masks.make_identity(nc, ident_f32[:])
        masks.make_identity(nc, ident_bf16[:])
        ident_u16 = ident_bf16
        mask_sb = consts.tile([QS, QS], f32)
        nc.sync.dma_start(mask_sb[:], mask[:, :])

        # ablation-only constant tiles (perf attribution experiments)
        const_kt = const_pt = None
        if no_ktp:
            const_kt_f32 = consts.tile([128, CHUNK], f32)
            nc.gpsimd.memset(const_kt_f32[:], 0.001)
            const_kt = consts.tile([128, CHUNK],
                                   f32r if s_fp32r else f32)
            nc.vector.tensor_copy(const_kt[:], const_kt_f32[:])
        if no_ptp:
            const_pt = consts.tile([128, CHUNK], bf16)
            nc.gpsimd.memset(const_pt[:], 0.001)

        # S-matmul operand dtype: float32r is a PE-rounded fp32 format that
        # streams at 1 cycle/row for moving dims >= 256 (vs 4 for fp32); the
        # producer (DVE copy) must write the rounded format itself.
        sdt = f32r if s_fp32r else f32

        def emit_all_bh():
            for bh in range(n_bh):
                # ---- Q^T for this bh ----
                q_nat = qt_pool.tile([QS, D], f32, tag="qnat")
                nc.sync.dma_start(q_nat[:], q[bh])
                qt_ps = kt_psum.tile([128, CHUNK], f32, tag="ktp")
                nc.tensor.transpose(qt_ps[:, 0:128], q_nat[:], ident_f32[:])
                qt = qt_pool.tile([D, QS], sdt, tag="qt")
                nc.vector.tensor_copy(qt[:], qt_ps[:, 0:128])

                ob = o_psum.tile([QS, D + 1], f32, tag="op")

                # chunk list: (k_src, v_src, row0, ncols, masked)
                ng = chunks_g if chunks_g is not None else kv_g // CHUNK
                ncc = chunks_c if chunks_c is not None else kv_c // CHUNK
                chunks = []
                for ci in range(ng):
                    chunks.append((kg, vg, (ci * CHUNK) % kv_g, CHUNK, False))
                for ci in range(ncc):
                    chunks.append((kc, vc, (ci * CHUNK) % kv_c, CHUNK, False))
                chunks.append((k, v, 0, QS, True))
                n_mm = sum((nc_ + 127) // 128 for _, _, _, nc_, _ in chunks)

                # group consecutive chunks from the same source into one
                # K DMA + one V DMA (bigger transfers, fewer SWDGE calls)
                groups = []
                for (ksrc, vsrc, row0, ncols, masked) in chunks:
                    g = groups[-1] if groups else None
                    if (g is not None and g[0] is ksrc and not masked
                            and not g[4] and g[2] + g[3] == row0
                            and g[3] + ncols <= dma_group * CHUNK):
                        groups[-1] = (g[0], g[1], g[2], g[3] + ncols, g[4])
                    else:
                        groups.append((ksrc, vsrc, row0, ncols, masked))

                mm_idx = 0
                chunk_i = 0
                GSUB = dma_group * SUB
                for (ksrc, vsrc, grow0, gncols, masked) in groups:
                    gnsub = (gncols + 127) // 128
                    vt = vt_pool.tile([128, GSUB, D + 1], bf16, tag="vt")
                    vsl = vsrc[bh, grow0:grow0 + gncols, :].rearrange(
                        "(c p) d -> p c d", p=128)
                    if v_mode == 'swdge':
                        nc.gpsimd.dma_start(vt[:, 0:gnsub, 0:D], vsl)
                    else:
                        # HWDGE fp32 load on the ACT ring + Pool cast to bf16
                        vf = vt_pool.tile([128, GSUB, D], f32, tag="vf")
                        nc.scalar.dma_start(vf[:, 0:gnsub, :], vsl)
                        if v_mode == 'hwdge':
                            nc.gpsimd.tensor_copy(vt[:, 0:gnsub, 0:D],
                                                  vf[:, 0:gnsub, :])
                    nc.gpsimd.memset(vt[:, 0:gnsub, D:D + 1], 1.0)
                    kgrp = k_group if k_group is not None else dma_group
                    kng = None
                    if kgrp > 1:
                        # one coarse K DMA for the whole group
                        kng = kn_pool.tile([128, GSUB, 128], f32, tag="kn")
                        ksl = ksrc[bh, grow0:grow0 + gncols, :].rearrange(
                            "(c p) d -> p c d", p=128)
                        nc.sync.dma_start(kng[:, 0:gnsub, :], ksl)
                    elif dma_only:
                        for off in range(0, gncols, CHUNK):
                            ncols = min(CHUNK, gncols - off)
                            nsub = (ncols + 127) // 128
                            kn_t = kn_pool.tile([128, SUB, 128], f32, tag="kn")
                            ksl = ksrc[bh, grow0 + off:grow0 + off + ncols,
                                       :].rearrange("(c p) d -> p c d", p=128)
                            eng = nc.sync if (k_rings == 1 or chunk_i % 2 == 0) \
                                else nc.scalar
                            eng.dma_start(kn_t[:, 0:nsub, :], ksl)
                            chunk_i += 1
                        continue
                    if dma_only:
                        chunk_i += (gncols + CHUNK - 1) // CHUNK
                        continue
                    for off in range(0, gncols, CHUNK):
                        ncols = min(CHUNK, gncols - off)
                        sub0 = off // 128

                        nsub = (ncols + 127) // 128
                        if kng is not None:
                            kn = kng[:, sub0:sub0 + nsub]
                        else:
                            # fine-grained per-chunk K DMA (keeps the
                            # transpose pipeline from waiting on a 1MB load);
                            # optionally alternate the two HWDGE rings
                            kn_t = kn_pool.tile([128, SUB, 128], f32, tag="kn")
                            ksl = ksrc[bh, grow0 + off:grow0 + off + ncols,
                                       :].rearrange("(c p) d -> p c d", p=128)
                            eng = nc.sync if (k_rings == 1 or chunk_i % 2 == 0) \
                                else nc.scalar
                            eng.dma_start(kn_t[:, 0:nsub, :], ksl)
                            kn = kn_t[:, 0:nsub]
                        vtc = vt[:, sub0:sub0 + nsub]
                        self_chunk_i = chunk_i
                        chunk_i += 1
                        if no_ktp:
                            kt = const_kt
                        elif kt_mode == 'u16':
                            # Exact fp32 K^T: two bf16-carrier transposes per
                            # subtile + one DVE interleave copy lo/hi -> fp32
                            kn16 = kn.bitcast(u16)     # [128, nsub, 256]
                            ktp = kt_psum.tile([128, 2, CHUNK], u16, tag="ktp")
                            for c in range(nsub):
                                for half in (0, 1):
                                    nc.tensor.transpose(
                                        ktp[:, half, c * 128:(c + 1) * 128],
                                        kn16[:, c, half::2], ident_u16[:])
                            kt = kt_pool.tile([128, CHUNK], sdt, tag="kt")
                            kt16 = kt[:].bitcast(u16).rearrange(
                                "p (kv two) -> p two kv", two=2)
                            nc.vector.tensor_copy(kt16[:, :, 0:ncols],
                                                  ktp[:, :, 0:ncols])
                        else:
                            # K^T via PE fp32 transpose -> PSUM -> SBUF (DVE)
                            ktp = kt_psum.tile([128, CHUNK], f32, tag="ktp")
                            for c in range(nsub):
                                nc.tensor.transpose(
                                    ktp[:, c * 128:(c + 1) * 128], kn[:, c, :],
                                    ident_f32[:])
                            kt = kt_pool.tile([128, CHUNK], sdt, tag="kt")
                            nc.vector.tensor_copy(kt[:, 0:ncols],
                                                  ktp[:, 0:ncols])

                        # S chunk
                        sp = s_psum.tile([QS, CHUNK], f32, tag="sp")
                        nc.tensor.matmul(sp[:, 0:ncols], qt[:], kt[:, 0:ncols],
                                         start=True, stop=True)
                        if masked:
                            nc.vector.tensor_add(sp[:, 0:ncols],
                                                 sp[:, 0:ncols], mask_sb[:])
                        # P = exp(S) -> bf16 SBUF
                        p = p_pool.tile([QS, CHUNK], bf16, tag="p")
                        nc.scalar.activation(p[:, 0:ncols], sp[:, 0:ncols], EXP)

                        # P^T via PE transpose (bf16) -> PSUM -> SBUF
                        if no_ptp:
                            pt = const_pt
                        else:
                            ptp = pt_psum.tile([128, CHUNK], bf16, tag="ptp")
                            for c in range(nsub):
                                nc.tensor.transpose(
                                    ptp[:, c * 128:(c + 1) * 128],
                                    p[:, c * 128:(c + 1) * 128], ident_bf16[:])
                            pt = pt_pool.tile([128, CHUNK], bf16, tag="pt")
                            # balance PSUM->SBUF copies between ACT and DVE
                            if self_chunk_i % 2 == 0:
                                nc.scalar.copy(pt[:, 0:ncols], ptp[:, 0:ncols])
                            else:
                                nc.vector.tensor_copy(pt[:, 0:ncols],
                                                      ptp[:, 0:ncols])

                        # PV accumulate into O [q, 129]
                        for c in range(nsub):
                            nc.tensor.matmul(
                                ob[:], pt[:, c * 128:(c + 1) * 128],
                                vtc[:, c, :],
                                start=(mm_idx == 0), stop=(mm_idx == n_mm - 1))
                            mm_idx += 1

                # ---- tail: divide by se and store ----
                o_sb = o_pool.tile([QS, D], f32, tag="o")
                if dma_only:
                    nc.vector.tensor_copy(o_sb[:], mask_sb[:])
                else:
                    recip = qt_pool.tile([QS, 1], f32, tag="recip")
                    nc.vector.reciprocal(recip[:], ob[:, D:D + 1])
                    nc.vector.tensor_scalar_mul(o_sb[:], ob[:, 0:D], recip[:])
                nc.sync.dma_start(y[bh], o_sb[:])

        for _ in range(repeat):
            emit_all_bh()


def build_bass(n_bh=BH_PER_CORE, kv_g=KV_G, kv_c=KV_C, s_fp32r=True, repeat=1,
               with_seed=False, kt_mode='pe', chunks_g=None, chunks_c=None,
               no_ktp=False, no_ptp=False, dma_only=False, v_mode='swdge',
               dma_group=4, k_group=1, k_rings=2, bufs_sbuf=4):
    import concourse.tile as tile
    from concourse import bacc, mybir

    f32 = mybir.dt.float32
    nc = bacc.Bacc("TRN2", target_bir_lowering=False, debug=False,
                   num_devices=N_CORES)

    def din(name, shape):
        return nc.dram_tensor(name, shape, f32, kind="ExternalInput").ap()

    seed = din('seed', [1, 1]) if with_seed else None
    ins = {
        'q': din('q', [n_bh, QS, D]),
        'k': din('k', [n_bh, QS, D]),
        'v': din('v', [n_bh, QS, D]),
        'kg': din('kg', [n_bh, kv_g, D]),
        'vg': din('vg', [n_bh, kv_g, D]),
        'kc': din('kc', [n_bh, kv_c, D]),
        'vc': din('vc', [n_bh, kv_c, D]),
        'mask': din('mask', [QS, QS]),
    }
    outs = {'y': nc.dram_tensor('y', [n_bh, QS, D], f32,
                                kind="ExternalOutput").ap()}

    with tile.TileContext(nc) as tc:
        if seed is not None:
            with tc.tile_pool(name="seedp", bufs=1) as seedp:
                st = seedp.tile([1, 1], f32)
                nc.sync.dma_start(st[:], seed[:, :])
        emit_attention(tc, outs, ins, n_bh=n_bh, kv_g=kv_g, kv_c=kv_c,
                       s_fp32r=s_fp32r, repeat=repeat, kt_mode=kt_mode,
                       chunks_g=chunks_g, chunks_c=chunks_c,
                       no_ktp=no_ktp, no_ptp=no_ptp, dma_only=dma_only,
                       v_mode=v_mode, dma_group=dma_group, k_group=k_group,
                       k_rings=k_rings, bufs_sbuf=bufs_sbuf)
    nc.compile()
    return nc


def build_proxy(s_fp32r=True, kt_mode='pe', repeat=1, **kw):
    """Perf-proxy: real instruction stream + real HBM byte volume, but the
    cache reads reuse one 512-row window so shipped inputs are tiny."""
    return build_bass(n_bh=BH_PER_CORE, kv_g=CHUNK, kv_c=CHUNK,
                      s_fp32r=s_fp32r, kt_mode=kt_mode, repeat=repeat,
                      chunks_g=KV_G // CHUNK, chunks_c=KV_C // CHUNK, **kw)


def proxy_inputs():
    rng = np.random.default_rng(0)
    f = lambda *s: rng.standard_normal(s, dtype=np.float32) * 0.1
    n = BH_PER_CORE
    one = {
        'q': f(n, QS, D), 'k': f(n, QS, D), 'v': f(n, QS, D),
        'kg': f(n, CHUNK, D), 'vg': f(n, CHUNK, D),
        'kc': f(n, CHUNK, D), 'vc': f(n, CHUNK, D),
        'mask': np.zeros((QS, QS), np.float32),
    }
    return [dict(one) for _ in range(N_CORES)]


def shard_inputs(q, k, v, k_cache_gpu, v_cache_gpu, k_cache_cpu, v_cache_cpu,
                 mask):
    in_maps = []
    for c in range(N_CORES):
        s = slice(c * BH_PER_CORE, (c + 1) * BH_PER_CORE)
        in_maps.append({
            'q': np.ascontiguousarray(q[s]),
            'k': np.ascontiguousarray(k[s]),
            'v': np.ascontiguousarray(v[s]),
            'kg': np.ascontiguousarray(k_cache_gpu[s]),
            'vg': np.ascontiguousarray(v_cache_gpu[s]),
            'kc': np.ascontiguousarray(k_cache_cpu[s]),
            'vc': np.ascontiguousarray(v_cache_cpu[s]),
            'mask': np.ascontiguousarray(mask),
        })
    return in_maps


def unshard_output(per_core_y):
    full = np.concatenate(per_core_y, axis=0)           # [BH, QS, D]
    out = full.reshape(BATCH, HEADS, QS, D).transpose(0, 2, 1, 3)
    return np.ascontiguousarray(out)


_NC_CACHE = {}


def kernel(q, k, v, k_cache_gpu, v_cache_gpu, k_cache_cpu, v_cache_cpu, mask):
    from concourse import bass_utils

    key = 'main'
    if key not in _NC_CACHE:
        _NC_CACHE[key] = build_bass()
    nc = _NC_CACHE[key]

    in_maps = shard_inputs(np.asarray(q, np.float32), np.asarray(k, np.float32),
                           np.asarray(v, np.float32),
                           np.asarray(k_cache_gpu, np.float32),
                           np.asarray(v_cache_gpu, np.float32),
                           np.asarray(k_cache_cpu, np.float32),
                           np.asarray(v_cache_cpu, np.float32),
                           np.asarray(mask, np.float32))
    res = bass_utils.run_bass_kernel_spmd(nc, in_maps,
                                          core_ids=list(range(N_CORES)))
    return unshard_output([r['y'] for r in res.results])



# revision 39
# speedup vs baseline: 1.2029x; 1.2029x over previous
"""Trainium2 Bass kernel for the two-branch KV-cache attention problem.

Math: the reference computes attention over [k_cache_gpu; k_new] (with a causal
mask on the new columns) and separately over k_cache_cpu, then merges the two
partial softmax states in log2-lse space.  That merge is mathematically the
softmax over the union of all kv columns, so we compute ONE softmax over all
4096 + 8192 + 128 = 12416 columns per (batch*head, q) row.  We use the
*unstable* softmax (no row-max subtraction): scores are N(0, sqrt(128)) so
exp stays well inside fp32 range, and masked (-65504) scores underflow to
exactly 0 like the reference.

Sharding: bh (=64) split across 8 cores, 8 bh each; merge is purely local.

Per 512-kv chunk (per bh):
  DMA   : K chunk fp32 -> SBUF [128,(4,128)]    (HWDGE)
          V chunk fp32 -> SBUF bf16 [128,(4,129)] (SWDGE cast) + ones column
  PE    : 4x fp32 transpose K tiles -> PSUM kt [d=128, kv=512]
  DVE   : copy kt PSUM -> SBUF (rounding to float32r when enabled)
  PE    : S = matmul(lhsT=Q^T, rhs=K^T) -> PSUM [q=128, kv=512]
  ACT   : P = exp(S) PSUM -> SBUF bf16
  PE    : 4x bf16 transpose P tiles -> PSUM pt [kv, q]
  ACT/DVE: copy pt PSUM -> SBUF (alternating)
  PE    : 4x matmul(lhsT=P^T tile, rhs=[V|1] bf16) accumulate -> O PSUM [q,129]
Tail per bh: recip(se=O[:,128]) on DVE, scale O[:,0:128], DMA out.

The ones column appended to V makes the PV matmul also produce the softmax
denominator (sum of the exact bf16-rounded P actually used for the numerator).
"""

import numpy as np

BATCH = 2
HEADS = 32
BH = BATCH * HEADS
QS = 128
D = 128
KV_G = 4096
KV_C = 8192
N_CORES = 8
BH_PER_CORE = BH // N_CORES

CHUNK = 512
SUB = CHUNK // 128  # 128-wide subtiles per chunk


def emit_attention(tc, outs, ins, n_bh=BH_PER_CORE, kv_g=KV_G, kv_c=KV_C,
                   s_fp32r=True, bufs_sbuf=3, repeat=1, kt_mode='pe',
                   chunks_g=None, chunks_c=None, no_ktp=False, no_ptp=False,
                   dma_only=False, v_mode='swdge', dma_group=1, k_group=None,
                   k_rings=1, psum_sp=2, psum_op=2):
    """Emit the attention program into TileContext `tc`."""
    from contextlib import ExitStack
    from concourse import masks, mybir

    nc = tc.nc
    f32 = mybir.dt.float32
    f32r = mybir.dt.float32r
    bf16 = mybir.dt.bfloat16
    EXP = mybir.ActivationFunctionType.Exp

    q, k, v = ins['q'], ins['k'], ins['v']
    kg, vg, kc, vc = ins['kg'], ins['vg'], ins['kc'], ins['vc']
    mask = ins['mask']
    y = outs['y']

    assert kv_g % CHUNK == 0 and kv_c % CHUNK == 0

    with ExitStack() as ctx:
        ep = ctx.enter_context

        consts = ep(tc.tile_pool(name="consts", bufs=1))
        kn_pool = ep(tc.tile_pool(name="kn", bufs=bufs_sbuf))
        vt_pool = ep(tc.tile_pool(name="vt", bufs=bufs_sbuf))
        kt_pool = ep(tc.tile_pool(name="kt", bufs=bufs_sbuf))
        p_pool = ep(tc.tile_pool(name="p", bufs=bufs_sbuf))
        pt_pool = ep(tc.tile_pool(name="pt", bufs=bufs_sbuf))
        qt_pool = ep(tc.tile_pool(name="qt", bufs=2))
        o_pool = ep(tc.tile_pool(name="o", bufs=2))
        kt_psum = ep(tc.tile_pool(name="ktp", bufs=2, space="PSUM"))
        s_psum = ep(tc.tile_pool(name="sp", bufs=psum_sp, space="PSUM"))
        pt_psum = ep(tc.tile_pool(name="ptp", bufs=2, space="PSUM"))
        o_psum = ep(tc.tile_pool(name="op", bufs=psum_op, space="PSUM"))

        u16 = bf16  # 2-byte carrier for bit-exact half transposes
        ident_f32 = consts.tile([128, 128], f32)
        ident_bf16 = consts.tile([128, 128], bf16)
        masks.make_identity(nc, ident_f32[:])
        masks.make_identity(nc, ident_bf16[:])
        ident_u16 = ident_bf16
        mask_sb = consts.tile([QS, QS], f32)
        nc.sync.dma_start(mask_sb[:], mask[:, :])

        # ablation-only constant tiles (perf attribution experiments)
        const_kt = const_pt = None
        if no_ktp:
            const_kt_f32 = consts.tile([128, CHUNK], f32)
            nc.gpsimd.memset(const_kt_f32[:], 0.001)
            const_kt = consts.tile([128, CHUNK],
                                   f32r if s_fp32r else f32)
            nc.vector.tensor_copy(const_kt[:], const_kt_f32[:])
        if no_ptp:
            const_pt = consts.tile([128, CHUNK], bf16)
            nc.gpsimd.memset(const_pt[:], 0.001)

        # S-matmul operand dtype: float32r is a PE-rounded fp32 format that
        # streams at 1 cycle/row for moving dims >= 256 (vs 4 for fp32); the
        # producer (DVE copy) must write the rounded format itself.
        sdt = f32r if s_fp32r else f32

        def emit_all_bh():
            for bh in range(n_bh):
                # ---- Q^T for this bh ----
                q_nat = qt_pool.tile([QS, D], f32, tag="qnat")
                nc.sync.dma_start(q_nat[:], q[bh])
                qt_ps = kt_psum.tile([128, CHUNK], f32, tag="ktp")
                nc.tensor.transpose(qt_ps[:, 0:128], q_nat[:], ident_f32[:])
                qt = qt_pool.tile([D, QS], sdt, tag="qt")
                nc.vector.tensor_copy(qt[:], qt_ps[:, 0:128])

                ob = o_psum.tile([QS, D + 1], f32, tag="op")

                # chunk list: (k_src, v_src, row0, ncols, masked)
                ng = chunks_g if chunks_g is not None else kv_g // CHUNK
                ncc = chunks_c if chunks_c is not None else kv_c // CHUNK
                chunks = []
                for ci in range(ng):
                    chunks.append((kg, vg, (ci * CHUNK) % kv_g, CHUNK, False))
                for ci in range(ncc):
                    chunks.append((kc, vc, (ci * CHUNK) % kv_c, CHUNK, False))
                chunks.append((k, v, 0, QS, True))
                n_mm = sum((nc_ + 127) // 128 for _, _, _, nc_, _ in chunks)

                # group consecutive chunks from the same source into one
                # K DMA + one V DMA (bigger transfers, fewer SWDGE calls)
                groups = []
                for (ksrc, vsrc, row0, ncols, masked) in chunks:
                    g = groups[-1] if groups else None
                    if (g is not None and g[0] is ksrc and not masked
                            and not g[4] and g[2] + g[3] == row0
                            and g[3] + ncols <= dma_group * CHUNK):
                        groups[-1] = (g[0], g[1], g[2], g[3] + ncols, g[4])
                    else:
                        groups.append((ksrc, vsrc, row0, ncols, masked))

                mm_idx = 0
                chunk_i = 0
                GSUB = dma_group * SUB
                for (ksrc, vsrc, grow0, gncols, masked) in groups:
                    gnsub = (gncols + 127) // 128
                    vt = vt_pool.tile([128, GSUB, D + 1], bf16, tag="vt")
                    vsl = vsrc[bh, grow0:grow0 + gncols, :].rearrange(
                        "(c p) d -> p c d", p=128)
                    if v_mode == 'swdge':
                        nc.gpsimd.dma_start(vt[:, 0:gnsub, 0:D], vsl)
                    else:
                        # HWDGE fp32 load on the ACT ring + Pool cast to bf16
                        vf = vt_pool.tile([128, GSUB, D], f32, tag="vf")
                        nc.scalar.dma_start(vf[:, 0:gnsub, :], vsl)
                        if v_mode == 'hwdge':
                            nc.gpsimd.tensor_copy(vt[:, 0:gnsub, 0:D],
                                                  vf[:, 0:gnsub, :])
                    nc.gpsimd.memset(vt[:, 0:gnsub, D:D + 1], 1.0)
                    kgrp = k_group if k_group is not None else dma_group
                    kng = None
                    if kgrp > 1:
                        # one coarse K DMA for the whole group
                        kng = kn_pool.tile([128, GSUB, 128], f32, tag="kn")
                        ksl = ksrc[bh, grow0:grow0 + gncols, :].rearrange(
                            "(c p) d -> p c d", p=128)
                        nc.sync.dma_start(kng[:, 0:gnsub, :], ksl)
                    elif dma_only:
                        for off in range(0, gncols, CHUNK):
                            ncols = min(CHUNK, gncols - off)
                            nsub = (ncols + 127) // 128
                            kn_t = kn_pool.tile([128, SUB, 128], f32, tag="kn")
                            ksl = ksrc[bh, grow0 + off:grow0 + off + ncols,
                                       :].rearrange("(c p) d -> p c d", p=128)
                            eng = nc.sync if (k_rings == 1 or chunk_i % 2 == 0) \
                                else nc.scalar
                            eng.dma_start(kn_t[:, 0:nsub, :], ksl)
                            chunk_i += 1
                        continue
                    if dma_only:
                        chunk_i += (gncols + CHUNK - 1) // CHUNK
                        continue
                    for off in range(0, gncols, CHUNK):
                        ncols = min(CHUNK, gncols - off)
                        sub0 = off // 128

                        nsub = (ncols + 127) // 128
                        if kng is not None:
                            kn = kng[:, sub0:sub0 + nsub]
                        else:
                            # fine-grained per-chunk K DMA (keeps the
                            # transpose pipeline from waiting on a 1MB load);
                            # optionally alternate the two HWDGE rings
                            kn_t = kn_pool.tile([128, SUB, 128], f32, tag="kn")
                            ksl = ksrc[bh, grow0 + off:grow0 + off + ncols,
                                       :].rearrange("(c p) d -> p c d", p=128)
                            eng = nc.sync if (k_rings == 1 or chunk_i % 2 == 0) \
                                else nc.scalar
                            eng.dma_start(kn_t[:, 0:nsub, :], ksl)
                            kn = kn_t[:, 0:nsub]
                        vtc = vt[:, sub0:sub0 + nsub]
                        self_chunk_i = chunk_i
                        chunk_i += 1
                        if no_ktp:
                            kt = const_kt
                        elif kt_mode == 'u16':
                            # Exact fp32 K^T: two bf16-carrier transposes per
                            # subtile + one DVE interleave copy lo/hi -> fp32
                            kn16 = kn.bitcast(u16)     # [128, nsub, 256]
                            ktp = kt_psum.tile([128, 2, CHUNK], u16, tag="ktp")
                            for c in range(nsub):
                                for half in (0, 1):
                                    nc.tensor.transpose(
                                        ktp[:, half, c * 128:(c + 1) * 128],
                                        kn16[:, c, half::2], ident_u16[:])
                            kt = kt_pool.tile([128, CHUNK], sdt, tag="kt")
                            kt16 = kt[:].bitcast(u16).rearrange(
                                "p (kv two) -> p two kv", two=2)
                            nc.vector.tensor_copy(kt16[:, :, 0:ncols],
                                                  ktp[:, :, 0:ncols])
                        else:
                            # K^T via PE fp32 transpose -> PSUM -> SBUF (DVE)
                            ktp = kt_psum.tile([128, CHUNK], f32, tag="ktp")
                            for c in range(nsub):
                                nc.tensor.transpose(
                                    ktp[:, c * 128:(c + 1) * 128], kn[:, c, :],
                                    ident_f32[:])
                            kt = kt_pool.tile([128, CHUNK], sdt, tag="kt")
                            nc.vector.tensor_copy(kt[:, 0:ncols],
                                                  ktp[:, 0:ncols])

                        # S chunk
                        sp = s_psum.tile([QS, CHUNK], f32, tag="sp")
                        nc.tensor.matmul(sp[:, 0:ncols], qt[:], kt[:, 0:ncols],
                                         start=True, stop=True)
                        if masked:
                            nc.vector.tensor_add(sp[:, 0:ncols],
                                                 sp[:, 0:ncols], mask_sb[:])
                        # P = exp(S) -> bf16 SBUF
                        p = p_pool.tile([QS, CHUNK], bf16, tag="p")
                        nc.scalar.activation(p[:, 0:ncols], sp[:, 0:ncols], EXP)

                        # P^T via PE transpose (bf16) -> PSUM -> SBUF
                        if no_ptp:
                            pt = const_pt
                        else:
                            ptp = pt_psum.tile([128, CHUNK], bf16, tag="ptp")
                            for c in range(nsub):
                                nc.tensor.transpose(
                                    ptp[:, c * 128:(c + 1) * 128],
                                    p[:, c * 128:(c + 1) * 128], ident_bf16[:])
                            pt = pt_pool.tile([128, CHUNK], bf16, tag="pt")
                            # balance PSUM->SBUF copies between ACT and DVE
                            if self_chunk_i % 2 == 0:
                                nc.scalar.copy(pt[:, 0:ncols], ptp[:, 0:ncols])
                            else:
                                nc.vector.tensor_copy(pt[:, 0:ncols],
                                                      ptp[:, 0:ncols])

                        # PV accumulate into O [q, 129]
                        for c in range(nsub):
                            nc.tensor.matmul(
                                ob[:], pt[:, c * 128:(c + 1) * 128],
                                vtc[:, c, :],
                                start=(mm_idx == 0), stop=(mm_idx == n_mm - 1))
                            mm_idx += 1

                # ---- tail: divide by se and store ----
                o_sb = o_pool.tile([QS, D], f32, tag="o")
                if dma_only:
                    nc.vector.tensor_copy(o_sb[:], mask_sb[:])
                else:
                    recip = qt_pool.tile([QS, 1], f32, tag="recip")
                    nc.vector.reciprocal(recip[:], ob[:, D:D + 1])
                    nc.vector.tensor_scalar_mul(o_sb[:], ob[:, 0:D], recip[:])
                nc.sync.dma_start(y[bh], o_sb[:])

        for _ in range(repeat):
            emit_all_bh()


def build_bass(n_bh=BH_PER_CORE, kv_g=KV_G, kv_c=KV_C, s_fp32r=True, repeat=1,
               with_seed=False, kt_mode='pe', chunks_g=None, chunks_c=None,
               no_ktp=False, no_ptp=False, dma_only=False, v_mode='swdge',
               dma_group=4, k_group=1, k_rings=2, bufs_sbuf=4, psum_sp=2,
               psum_op=2):
    import concourse.tile as tile
    from concourse import bacc, mybir

    f32 = mybir.dt.float32
    nc = bacc.Bacc("TRN2", target_bir_lowering=False, debug=False,
                   num_devices=N_CORES)

    def din(name, shape):
        return nc.dram_tensor(name, shape, f32, kind="ExternalInput").ap()

    seed = din('seed', [1, 1]) if with_seed else None
    ins = {
        'q': din('q', [n_bh, QS, D]),
        'k': din('k', [n_bh, QS, D]),
        'v': din('v', [n_bh, QS, D]),
        'kg': din('kg', [n_bh, kv_g, D]),
        'vg': din('vg', [n_bh, kv_g, D]),
        'kc': din('kc', [n_bh, kv_c, D]),
        'vc': din('vc', [n_bh, kv_c, D]),
        'mask': din('mask', [QS, QS]),
    }
    outs = {'y': nc.dram_tensor('y', [n_bh, QS, D], f32,
                                kind="ExternalOutput").ap()}

    with tile.TileContext(nc) as tc:
        if seed is not None:
            with tc.tile_pool(name="seedp", bufs=1) as seedp:
                st = seedp.tile([1, 1], f32)
                nc.sync.dma_start(st[:], seed[:, :])
        emit_attention(tc, outs, ins, n_bh=n_bh, kv_g=kv_g, kv_c=kv_c,
                       s_fp32r=s_fp32r, repeat=repeat, kt_mode=kt_mode,
                       chunks_g=chunks_g, chunks_c=chunks_c,
                       no_ktp=no_ktp, no_ptp=no_ptp, dma_only=dma_only,
                       v_mode=v_mode, dma_group=dma_group, k_group=k_group,
                       k_rings=k_rings, bufs_sbuf=bufs_sbuf, psum_sp=psum_sp,
                       psum_op=psum_op)
    nc.compile()
    return nc


def build_proxy(s_fp32r=True, kt_mode='pe', repeat=1, **kw):
    """Perf-proxy: real instruction stream + real HBM byte volume, but the
    cache reads reuse one 512-row window so shipped inputs are tiny."""
    return build_bass(n_bh=BH_PER_CORE, kv_g=CHUNK, kv_c=CHUNK,
                      s_fp32r=s_fp32r, kt_mode=kt_mode, repeat=repeat,
                      chunks_g=KV_G // CHUNK, chunks_c=KV_C // CHUNK, **kw)


def proxy_inputs():
    rng = np.random.default_rng(0)
    f = lambda *s: rng.standard_normal(s, dtype=np.float32) * 0.1
    n = BH_PER_CORE
    one = {
        'q': f(n, QS, D), 'k': f(n, QS, D), 'v': f(n, QS, D),
        'kg': f(n, CHUNK, D), 'vg': f(n, CHUNK, D),
        'kc': f(n, CHUNK, D), 'vc': f(n, CHUNK, D),
        'mask': np.zeros((QS, QS), np.float32),
    }
    return [dict(one) for _ in range(N_CORES)]


def shard_inputs(q, k, v, k_cache_gpu, v_cache_gpu, k_cache_cpu, v_cache_cpu,
                 mask):
    in_maps = []
    for c in range(N_CORES):
        s = slice(c * BH_PER_CORE, (c + 1) * BH_PER_CORE)
        in_maps.append({
            'q': np.ascontiguousarray(q[s]),
            'k': np.ascontiguousarray(k[s]),
            'v': np.ascontiguousarray(v[s]),
            'kg': np.ascontiguousarray(k_cache_gpu[s]),
            'vg': np.ascontiguousarray(v_cache_gpu[s]),
            'kc': np.ascontiguousarray(k_cache_cpu[s]),
            'vc': np.ascontiguousarray(v_cache_cpu[s]),
            'mask': np.ascontiguousarray(mask),
        })
    return in_maps


def unshard_output(per_core_y):
    full = np.concatenate(per_core_y, axis=0)           # [BH, QS, D]
    out = full.reshape(BATCH, HEADS, QS, D).transpose(0, 2, 1, 3)
    return np.ascontiguousarray(out)


_NC_CACHE = {}


def kernel(q, k, v, k_cache_gpu, v_cache_gpu, k_cache_cpu, v_cache_cpu, mask):
    from concourse import bass_utils

    key = 'main'
    if key not in _NC_CACHE:
        _NC_CACHE[key] = build_bass()
    nc = _NC_CACHE[key]

    in_maps = shard_inputs(np.asarray(q, np.float32), np.asarray(k, np.float32),
                           np.asarray(v, np.float32),
                           np.asarray(k_cache_gpu, np.float32),
                           np.asarray(v_cache_gpu, np.float32),
                           np.asarray(k_cache_cpu, np.float32),
                           np.asarray(v_cache_cpu, np.float32),
                           np.asarray(mask, np.float32))
    res = bass_utils.run_bass_kernel_spmd(nc, in_maps,
                                          core_ids=list(range(N_CORES)))
    return unshard_output([r['y'] for r in res.results])

